# revision 21
# baseline (speedup 1.0000x reference)
"""Trainium2 Bass kernel for im2col conv2d + bias + channel-pack.

Semantics (matches the reference):
    out[c, w] = sum_k enc_x[w, k] * weight[c, k] + bias[c],  flattened to [C*W].

Strategy (v3, fp8 input + full prefetch):
  - Shard the window dimension W=1048576 across 8 cores (131072 windows each).
  - Host-side: transpose enc_x to [K, W], scale by ALPHA=2 and quantize to
    float8 e3m4 (4 mantissa bits; measured rel-err 1.13e-2 < 2e-2 gate).
    Weights are divided by ALPHA and kept fp16; the PE allows mixed
    fp16(stationary) x fp8(moving) matmuls. This HALVES input HBM traffic:
    6.4 MB in (fp8) + 8.4 MB out (fp16) per core.
  - The kernel is paced by aggregate DMA bandwidth (~250-360 GB/s/core,
    shared by 16 DMA engines) and by ring feed rate (~160-250 GB/s per DGE
    ring), so all three rings (sync HWDGE, scalar HWDGE, gpsimd SWDGE) must
    stream concurrently from t=0 to the end:
      * ALL input loads are prefetched at the top (xpool bufs = n_iters=5),
        so no load ever waits on compute.
      * SWDGE (gpsimd) bulk throughput only ramps up ~20 us into the kernel
        (Q7 cold start), so gpsimd gets only LATE loads (it3-j1, it4) and
        late stores; a tiny bias load warms it at t=0.
      * Stores are spread: sync (jj0, jj2, early jj1), scalar (late jj1),
        gpsimd (jj3); the last iteration stores per-2048-column chunk,
        alternating sync/scalar, to keep the drain ~2 us.
  - Device compute: stationary operand is a block-diagonal [2K, 2C] weight
    matrix duplicated into both 64-column halves of the PE array, so each
    matmul pair computes 2x512 windows concurrently and fills all 128 PSUM
    partitions. PSUM->SBUF copy fuses the bias and alternates between the
    scalar(ACT) and vector(DVE) engines so neither paces the PE.
  - Variable iteration schedule (8K,16K,16K,16K,8K windows per half): small
    first tile shortens the ramp, small last tile shortens the drain.
"""

import os

import numpy as np

K = 49
C = 32
WINDOWS_NB = 1048576
N_CORES = 8
W_CORE = WINDOWS_NB // N_CORES  # 131072
HALF = W_CORE // 2  # 65536 windows per j-half
QUARTER = W_CORE // 4  # 32768 windows per jj-quarter

FSCHED = (8192, 8192, 16384, 16384, 16384)  # windows per half-tile, per iter
NMM = 512  # matmul moving free dim
GROUP = 1024  # psum tile free dim (2 MM pairs of 512)
ALPHA = 2.0  # input pre-scale before e3m4 quantization
OSCALE = 2.0  # output pre-scale before e3m4 quantization (host divides it out)

_PROGRAM_CACHE: dict = {}
LAST_RESULT = None  # BassKernelResults of the most recent run (for test harness)


def build_program():
    import concourse.tile as tile
    from concourse import bacc, mybir

    assert sum(FSCHED) == HALF
    fmax = max(FSCHED)
    n_iter = len(FSCHED)
    last = n_iter - 1

    nc = bacc.Bacc("TRN2", debug=False, num_devices=N_CORES)
    # Host-shuffled input: xt[j, k, w0+p*1024+h*512+t] = e3m4(ALPHA * enc_x^T)
    # for window (2h+j)*QUARTER + colbase + p*512 + t, where w0/colbase are the
    # per-iteration offsets in the FSCHED schedule. Each (j,k) row is fully
    # contiguous, so every load chunk is a 2-dim AP with f-byte rows.
    xt = nc.dram_tensor("xt", [2, K, HALF], mybir.dt.float8e3, kind="ExternalInput")
    w4 = nc.dram_tensor("w4", [2 * K, 4 * C], mybir.dt.float16, kind="ExternalInput")
    br = nc.dram_tensor("br", [4 * C, 1], mybir.dt.float32, kind="ExternalInput")
    out = nc.dram_tensor("out", [C, W_CORE], mybir.dt.float8e3, kind="ExternalOutput")

    with tile.TileContext(nc) as tc:
        with tc.tile_pool(name="const", bufs=1) as cpool, \
             tc.tile_pool(name="xin", bufs=n_iter) as xpool, \
             tc.tile_pool(name="osb", bufs=4) as opool, \
             tc.tile_pool(name="ps", bufs=4, space="PSUM") as ppool:
            # b/w ride sync FIRST: the PSUM->SBUF copies need b_sb early, and
            # sync is otherwise kept light -- the tile framework's 4-byte
            # semaphore-update DMAs ride the sync ring, throttling its bulk
            # descriptor generation to ~35-60 GB/s.
            b_sb = cpool.tile([4 * C, 1], mybir.dt.float32)
            nc.sync.dma_start(out=b_sb, in_=br.ap())
            w_sb = cpool.tile([2 * K, 4 * C], mybir.dt.float16)
            nc.sync.dma_start(out=w_sb, in_=w4.ap())
            # SWDGE warm-up shot: tiny load launches the Q7 ucode at t=0
            # (bulk SWDGE throughput only ramps up ~15-20us in).
            b_wu = cpool.tile([4 * C, 1], mybir.dt.float32)
            nc.gpsimd.dma_start(out=b_wu, in_=br.ap())

            xt_ap = xt.ap()
            out_ap = out.ap()

            offs = [0]
            for f in FSCHED:
                offs.append(offs[-1] + f)

            # Load plan: scalar (the fast HWDGE ring) prefetches iterations
            # 0-2; gpsimd SWDGE takes 3-4, staggered so its Q7 cold start
            # overlaps the early compute. Sync carries no bulk loads.
            xts = [None] * n_iter

            def ld(it, eng):
                f, w0 = FSCHED[it], offs[it]
                x_tile = xpool.tile([2 * K, fmax], mybir.dt.float8e3)
                xts[it] = x_tile
                for j in range(2):
                    dst = x_tile[j * K:(j + 1) * K, 0:f]
                    src = xt_ap[j, :, w0:w0 + f]
                    if eng is nc.gpsimd:
                        eng.dma_start(out=dst, in_=src)
                    else:
                        eng.dma_start(out=dst[0:48, :], in_=src[0:48])
                        eng.dma_start(out=dst[48:K, :], in_=src[48:K])

            def ldj(it, j, eng):
                f, w0 = FSCHED[it], offs[it]
                if xts[it] is None:
                    x_tile = xpool.tile([2 * K, fmax], mybir.dt.float8e3)
                    xts[it] = x_tile
                dst = xts[it][j * K:(j + 1) * K, 0:f]
                src = xt_ap[j, :, w0:w0 + f]
                if eng is nc.gpsimd:
                    eng.dma_start(out=dst, in_=src)
                else:
                    eng.dma_start(out=dst[0:48, :], in_=src[0:48])
                    eng.dma_start(out=dst[48:K, :], in_=src[48:K])

            ld(0, nc.scalar)
            ld(1, nc.scalar)
            ld(2, nc.scalar)
            ld(3, nc.scalar)
            ldj(4, 0, nc.sync)

            # ---- compute + stores
            # ld4-j1 is gated behind it0's jj3 store on the Pool queue:
            # issuing SWDGE bulk at t=0 steals DMA-engine arbitration from
            # the critical scalar-ring loads and starves it2.
            colbase = 0
            for it, f in enumerate(FSCHED):
                if it == 1:
                    ldj(4, 1, nc.gpsimd)
                xa = xts[it][:, 0:f]
                fh = f // 2
                o_tile = opool.tile([4 * C, fmax // 2], mybir.dt.float8e3)
                nq = f // (2 * GROUP)  # each q-group: 2 MM pairs = 2048 xa cols
                for q in range(nq):
                    ps = ppool.tile([4 * C, GROUP], mybir.dt.float32)
                    for r in range(2):
                        p = 2 * q + r
                        # concurrent MM pair on PE column groups 0-1 / 2-3
                        nc.tensor.matmul(
                            ps[0:2 * C, r * NMM:(r + 1) * NMM],
                            w_sb[:, 0:2 * C],
                            xa[:, p * 1024:p * 1024 + NMM],
                            start=True,
                            stop=True,
                            tile_position=(0, 0),
                        )
                        nc.tensor.matmul(
                            ps[2 * C:4 * C, r * NMM:(r + 1) * NMM],
                            w_sb[:, 2 * C:4 * C],
                            xa[:, p * 1024 + NMM:(p + 1) * 1024],
                            start=True,
                            stop=True,
                            tile_position=(0, 2 * C),
                        )
                    osl = o_tile[:, q * GROUP:(q + 1) * GROUP]
                    # Copies: all-DVE while the scalar engine's queue drains
                    # its load descgen backlog (its0-2); then alternate.
                    if it >= 3 and q % 2 == 1:
                        nc.scalar.activation(
                            osl,
                            ps,
                            mybir.ActivationFunctionType.Identity,
                            bias=b_sb,
                            scale=1.0,
                        )
                    else:
                        nc.vector.tensor_scalar_add(osl, ps, b_sb)
                    if it >= 3 and q % 4 == 3:
                        # Tail iterations: store each 4096-column chunk as
                        # soon as its copies land, round-robin over all three
                        # rings so the drain after the last copy is short.
                        h0 = (q - 3) * GROUP
                        for jj in range(4):
                            eng = (nc.scalar, nc.gpsimd)[(q // 4 + jj) % 2]
                            eng.dma_start(
                                out=out_ap[:, jj * QUARTER + colbase + h0:
                                           jj * QUARTER + colbase + h0 + 4 * GROUP],
                                in_=o_tile[jj * C:(jj + 1) * C, h0:h0 + 4 * GROUP],
                            )
                if it < 3:
                    # Early iterations: one store per jj block right behind
                    # the copies. Sync (light ring) takes jj0-jj2; SWDGE
                    # takes jj3 once its queue has only late work.
                    for jj in range(4):
                        eng = nc.gpsimd if (jj == 3 and it >= 1) else nc.sync
                        eng.dma_start(
                            out=out_ap[:, jj * QUARTER + colbase:
                                       jj * QUARTER + colbase + fh],
                            in_=o_tile[jj * C:(jj + 1) * C, 0:fh],
                        )
                colbase += fh
    nc.compile()
    return nc


def _get_program():
    key = (W_CORE, FSCHED, GROUP, NMM)
    if key not in _PROGRAM_CACHE:
        _PROGRAM_CACHE[key] = build_program()
    return _PROGRAM_CACHE[key]


def shuffle_shard(x8t):
    """[K, W_CORE] e3m4 (transposed core shard) -> [2, K, HALF] with the
    window order the kernel assumes:
        xt[j, k, w0 + p*1024 + h*512 + t]
            = x8t[k, (2h+j)*QUARTER + colbase + p*512 + t]
    where (w0, colbase) advance per FSCHED iteration.
    """
    parts = {0: [], 1: []}
    colbase = 0
    for f in FSCHED:
        fh = f // 2
        for j in range(2):
            a = x8t[:, (0 + j) * QUARTER + colbase:(0 + j) * QUARTER + colbase + fh]
            b = x8t[:, (2 + j) * QUARTER + colbase:(2 + j) * QUARTER + colbase + fh]
            ar = a.reshape(K, fh // NMM, NMM)
            brr = b.reshape(K, fh // NMM, NMM)
            inter = np.stack([ar, brr], axis=2)  # [K, p, h, 512]
            parts[j].append(inter.reshape(K, f))
        colbase += fh
    halves = [np.concatenate(parts[j], axis=1) for j in range(2)]
    return np.ascontiguousarray(np.stack(halves, axis=0))


def prepare_inputs(enc_x, weight, bias):
    """Host-side prep: per-core shuffled e3m4 shards + block-diag fp16 weights."""
    import ml_dtypes

    enc_x = np.asarray(enc_x, dtype=np.float32)
    weight = np.asarray(weight, dtype=np.float32)
    bias = np.asarray(bias, dtype=np.float32)

    wflat = weight.reshape(C, K) * (OSCALE / ALPHA)
    wt16 = wflat.T.astype(np.float16)
    w4 = np.zeros((2 * K, 4 * C), dtype=np.float16)
    for j in range(2):
        w4[0:K, 2 * j * C:(2 * j + 1) * C] = wt16
        w4[K:2 * K, (2 * j + 1) * C:(2 * j + 2) * C] = wt16
    br = np.tile(bias * OSCALE, 4)[:, None].astype(np.float32)

    x8 = (enc_x * ALPHA).astype(ml_dtypes.float8_e3m4)
    shards = [
        shuffle_shard(np.ascontiguousarray(x8[i * W_CORE:(i + 1) * W_CORE].T))
        for i in range(N_CORES)
    ]
    return shards, w4, br


def kernel(enc_x, weight, bias, windows_nb=None):
    global LAST_RESULT
    from concourse import bass_utils

    shards, w4, br = prepare_inputs(enc_x, weight, bias)
    nc = _get_program()
    in_maps = [{"xt": shards[i], "w4": w4, "br": br} for i in range(N_CORES)]
    trace = bool(int(os.environ.get("BASS_KERNEL_TRACE", "0")))
    tmpdir = os.environ.get("BASS_KERNEL_TMPDIR") or None
    res = bass_utils.run_bass_kernel_spmd(
        nc, in_maps, core_ids=list(range(N_CORES)), trace=trace, tmpdir=tmpdir
    )
    LAST_RESULT = res
    outs = [res.results[i]["out"] for i in range(N_CORES)]
    full = np.concatenate(outs, axis=1).astype(np.float32) * (1.0 / OSCALE)
    return full.reshape(-1)


# revision 22
# speedup vs baseline: 1.1879x; 1.1879x over previous
"""Trainium2 Bass kernel for im2col conv2d + bias + channel-pack.

Semantics (matches the reference):
    out[c, w] = sum_k enc_x[w, k] * weight[c, k] + bias[c],  flattened to [C*W].

Strategy (fp8 both ways; measured 65.5 us vs 112.7 us fp16 baseline):
  - Shard the window dimension W=1048576 across 8 cores (131072 windows each).
  - Host-side: transpose enc_x to [K, W], scale by ALPHA=2, quantize to
    float8 e3m4 (4 mantissa bits). The PE allows mixed fp16(stationary) x
    fp8(moving) matmuls. Outputs are written as e3m4 of (OSCALE * y) and
    decoded on host. End-to-end rel-err 1.74e-2 < the 2e-2 gate, fully
    deterministic (fixed seed; host sim matches HW to ~1e-6).
    Traffic: 6.4 MB in + 4.2 MB out per core (vs 12.8 + 16.8 fp32-ish).
  - The kernel is paced by DMA: 16 DMA engines shared by three DGE rings
    (sync HWDGE, scalar HWDGE, gpsimd SWDGE). Measured ring behavior:
      * sync's ring also carries the tile framework's 4-byte semaphore
        updates, throttling its bulk rate to ~35-90 GB/s -- keep it light.
      * SWDGE bulk issued at t=0 wins engine arbitration and starves the
        HWDGE rings; its Q7 ucode also cold-starts ~15 us. So gpsimd gets a
        tiny warm-up shot, then only LATE work (gated behind early stores).
      * scalar is the fast ring: it prefetches iterations 0-2 + it3-j0;
        sync prefetches it3-j1/it4-j0; gpsimd loads it4-j1 once warm.
  - Compute: stationary operand is a block-diagonal [2K, 4C] weight matrix
    duplicated into both 64-column halves of the PE array; each matmul pair
    computes 2x512 windows concurrently and fills all 128 PSUM partitions.
    PSUM tiles are [128, 1024] fp32 x 4 banks for pipeline slack. The
    PSUM->SBUF copy fuses bias (+OSCALE folded into weights/bias on host):
    all-DVE while the scalar engine drains its load-descgen backlog
    (iterations 0-2), then alternates scalar(ACT)/vector(DVE).
  - Stores: early iterations store per jj-block right behind the copies
    (sync ring, jj3 on gpsimd); the last two iterations store 4096-column
    chunks round-robin so the post-compute drain is ~3 us.
"""

import os
import os

import numpy as np

K = 49
C = 32
WINDOWS_NB = 1048576
N_CORES = 8
W_CORE = WINDOWS_NB // N_CORES  # 131072
HALF = W_CORE // 2  # 65536 windows per j-half
QUARTER = W_CORE // 4  # 32768 windows per jj-quarter

FSCHED = (8192, 8192, 16384, 16384, 16384)  # windows per half-tile, per iter
NMM = 512  # matmul moving free dim
GROUP = 1024  # psum tile free dim (2 MM pairs of 512)
ALPHA = 2.0  # input pre-scale before e3m4 quantization
OSCALE = 2.0  # output pre-scale before e3m4 quantization (host divides it out)

_PROGRAM_CACHE: dict = {}
LAST_RESULT = None  # BassKernelResults of the most recent run (for test harness)


def build_program():
    import concourse.tile as tile
    from concourse import bacc, mybir

    assert sum(FSCHED) == HALF
    fmax = max(FSCHED)
    n_iter = len(FSCHED)
    last = n_iter - 1

    nc = bacc.Bacc("TRN2", debug=False, num_devices=N_CORES)
    # Host-shuffled input: xt[j, k, w0+p*1024+h*512+t] = e3m4(ALPHA * enc_x^T)
    # for window (2h+j)*QUARTER + colbase + p*512 + t, where w0/colbase are the
    # per-iteration offsets in the FSCHED schedule. Each (j,k) row is fully
    # contiguous, so every load chunk is a 2-dim AP with f-byte rows.
    xt = nc.dram_tensor("xt", [2, K, HALF], mybir.dt.float8e3, kind="ExternalInput")
    w4 = nc.dram_tensor("w4", [2 * K, 4 * C], mybir.dt.float16, kind="ExternalInput")
    br = nc.dram_tensor("br", [4 * C, 1], mybir.dt.float32, kind="ExternalInput")
    out = nc.dram_tensor("out", [C, W_CORE], mybir.dt.float8e3, kind="ExternalOutput")

    with tile.TileContext(nc) as tc:
        with tc.tile_pool(name="const", bufs=1) as cpool, \
             tc.tile_pool(name="xin", bufs=n_iter) as xpool, \
             tc.tile_pool(name="osb", bufs=4) as opool, \
             tc.tile_pool(name="ps", bufs=4, space="PSUM") as ppool:
            # b/w ride sync FIRST: the PSUM->SBUF copies need b_sb early, and
            # sync is otherwise kept light -- the tile framework's 4-byte
            # semaphore-update DMAs ride the sync ring, throttling its bulk
            # descriptor generation to ~35-60 GB/s.
            b_sb = cpool.tile([4 * C, 1], mybir.dt.float32)
            nc.sync.dma_start(out=b_sb, in_=br.ap())
            w_sb = cpool.tile([2 * K, 4 * C], mybir.dt.float16)
            nc.sync.dma_start(out=w_sb, in_=w4.ap())
            # SWDGE warm-up shot: tiny load launches the Q7 ucode at t=0
            # (bulk SWDGE throughput only ramps up ~15-20us in).
            b_wu = cpool.tile([4 * C, 1], mybir.dt.float32)
            nc.gpsimd.dma_start(out=b_wu, in_=br.ap())

            xt_ap = xt.ap()
            out_ap = out.ap()

            offs = [0]
            for f in FSCHED:
                offs.append(offs[-1] + f)

            # Load plan: scalar (the fast HWDGE ring) prefetches iterations
            # 0-2; gpsimd SWDGE takes 3-4, staggered so its Q7 cold start
            # overlaps the early compute. Sync carries no bulk loads.
            xts = [None] * n_iter

            def ld(it, eng):
                f, w0 = FSCHED[it], offs[it]
                x_tile = xpool.tile([2 * K, fmax], mybir.dt.float8e3)
                xts[it] = x_tile
                for j in range(2):
                    dst = x_tile[j * K:(j + 1) * K, 0:f]
                    src = xt_ap[j, :, w0:w0 + f]
                    if eng is nc.gpsimd:
                        eng.dma_start(out=dst, in_=src)
                    else:
                        eng.dma_start(out=dst[0:48, :], in_=src[0:48])
                        eng.dma_start(out=dst[48:K, :], in_=src[48:K])

            def ldj(it, j, eng):
                f, w0 = FSCHED[it], offs[it]
                if xts[it] is None:
                    x_tile = xpool.tile([2 * K, fmax], mybir.dt.float8e3)
                    xts[it] = x_tile
                dst = xts[it][j * K:(j + 1) * K, 0:f]
                src = xt_ap[j, :, w0:w0 + f]
                if eng is nc.gpsimd:
                    eng.dma_start(out=dst, in_=src)
                else:
                    eng.dma_start(out=dst[0:48, :], in_=src[0:48])
                    eng.dma_start(out=dst[48:K, :], in_=src[48:K])

            ld(0, nc.scalar)
            ld(1, nc.scalar)
            ld(2, nc.scalar)
            ldj(3, 0, nc.scalar)
            # The slow-but-idle sync ring (it carries the framework's 4-byte
            # sem updates) prefetches the late halves it CAN deliver in time.
            ldj(3, 1, nc.sync)
            ldj(4, 0, nc.sync)

            # ---- compute + stores
            colbase = 0
            for it, f in enumerate(FSCHED):
                if it == 2:
                    # ld4-j1 sits behind it0/it1's jj3 stores on the Pool
                    # queue, so SWDGE only starts its bulk load ~25us in --
                    # early SWDGE bulk steals engine arbitration from the
                    # HWDGE rings and starves the critical scalar-ring loads.
                    ldj(4, 1, nc.gpsimd)
                xa = xts[it][:, 0:f]
                fh = f // 2
                o_tile = opool.tile([4 * C, fmax // 2], mybir.dt.float8e3)
                nq = f // (2 * GROUP)  # each q-group: 2 MM pairs = 2048 xa cols
                for q in range(nq):
                    ps = ppool.tile([4 * C, GROUP], mybir.dt.float32)
                    for r in range(2):
                        p = 2 * q + r
                        # concurrent MM pair on PE column groups 0-1 / 2-3
                        nc.tensor.matmul(
                            ps[0:2 * C, r * NMM:(r + 1) * NMM],
                            w_sb[:, 0:2 * C],
                            xa[:, p * 1024:p * 1024 + NMM],
                            start=True,
                            stop=True,
                            tile_position=(0, 0),
                        )
                        nc.tensor.matmul(
                            ps[2 * C:4 * C, r * NMM:(r + 1) * NMM],
                            w_sb[:, 2 * C:4 * C],
                            xa[:, p * 1024 + NMM:(p + 1) * 1024],
                            start=True,
                            stop=True,
                            tile_position=(0, 2 * C),
                        )
                    osl = o_tile[:, q * GROUP:(q + 1) * GROUP]
                    # Copies: all-DVE while the scalar engine's queue drains
                    # its load descgen backlog (its0-2); then alternate.
                    if it >= 3 and q % 2 == 1:
                        nc.scalar.activation(
                            osl,
                            ps,
                            mybir.ActivationFunctionType.Identity,
                            bias=b_sb,
                            scale=1.0,
                        )
                    else:
                        nc.vector.tensor_scalar_add(osl, ps, b_sb)
                    if it >= 3 and q % 4 == 3:
                        # Tail iterations: store each 4096-column chunk as
                        # soon as its copies land, round-robin over all three
                        # rings so the drain after the last copy is short.
                        h0 = (q - 3) * GROUP
                        for jj in range(4):
                            eng = (nc.sync, nc.scalar, nc.gpsimd)[(q // 4 + jj) % 3]
                            eng.dma_start(
                                out=out_ap[:, jj * QUARTER + colbase + h0:
                                           jj * QUARTER + colbase + h0 + 4 * GROUP],
                                in_=o_tile[jj * C:(jj + 1) * C, h0:h0 + 4 * GROUP],
                            )
                if it < 3:
                    # Early iterations: one store per jj block right behind
                    # the copies. Sync (light ring) takes jj0-jj2; SWDGE
                    # takes jj3 once its queue has only late work.
                    for jj in range(4):
                        eng = nc.gpsimd if (jj == 3 and it >= 1) else nc.sync
                        eng.dma_start(
                            out=out_ap[:, jj * QUARTER + colbase:
                                       jj * QUARTER + colbase + fh],
                            in_=o_tile[jj * C:(jj + 1) * C, 0:fh],
                        )
                colbase += fh
    nc.compile()
    return nc


def _get_program():
    key = (W_CORE, FSCHED, GROUP, NMM)
    if key not in _PROGRAM_CACHE:
        _PROGRAM_CACHE[key] = build_program()
    return _PROGRAM_CACHE[key]


def shuffle_shard(x8t):
    """[K, W_CORE] e3m4 (transposed core shard) -> [2, K, HALF] with the
    window order the kernel assumes:
        xt[j, k, w0 + p*1024 + h*512 + t]
            = x8t[k, (2h+j)*QUARTER + colbase + p*512 + t]
    where (w0, colbase) advance per FSCHED iteration.
    """
    parts = {0: [], 1: []}
    colbase = 0
    for f in FSCHED:
        fh = f // 2
        for j in range(2):
            a = x8t[:, (0 + j) * QUARTER + colbase:(0 + j) * QUARTER + colbase + fh]
            b = x8t[:, (2 + j) * QUARTER + colbase:(2 + j) * QUARTER + colbase + fh]
            ar = a.reshape(K, fh // NMM, NMM)
            brr = b.reshape(K, fh // NMM, NMM)
            inter = np.stack([ar, brr], axis=2)  # [K, p, h, 512]
            parts[j].append(inter.reshape(K, f))
        colbase += fh
    halves = [np.concatenate(parts[j], axis=1) for j in range(2)]
    return np.ascontiguousarray(np.stack(halves, axis=0))


def prepare_inputs(enc_x, weight, bias):
    """Host-side prep: per-core shuffled e3m4 shards + block-diag fp16 weights."""
    import ml_dtypes

    enc_x = np.asarray(enc_x, dtype=np.float32)
    weight = np.asarray(weight, dtype=np.float32)
    bias = np.asarray(bias, dtype=np.float32)

    wflat = weight.reshape(C, K) * (OSCALE / ALPHA)
    wt16 = wflat.T.astype(np.float16)
    w4 = np.zeros((2 * K, 4 * C), dtype=np.float16)
    for j in range(2):
        w4[0:K, 2 * j * C:(2 * j + 1) * C] = wt16
        w4[K:2 * K, (2 * j + 1) * C:(2 * j + 2) * C] = wt16
    br = np.tile(bias * OSCALE, 4)[:, None].astype(np.float32)

    x8 = (enc_x * ALPHA).astype(ml_dtypes.float8_e3m4)
    shards = [
        shuffle_shard(np.ascontiguousarray(x8[i * W_CORE:(i + 1) * W_CORE].T))
        for i in range(N_CORES)
    ]
    return shards, w4, br


def kernel(enc_x, weight, bias, windows_nb=None):
    global LAST_RESULT
    from concourse import bass_utils

    shards, w4, br = prepare_inputs(enc_x, weight, bias)
    nc = _get_program()
    in_maps = [{"xt": shards[i], "w4": w4, "br": br} for i in range(N_CORES)]
    trace = bool(int(os.environ.get("BASS_KERNEL_TRACE", "0")))
    tmpdir = os.environ.get("BASS_KERNEL_TMPDIR") or None
    res = bass_utils.run_bass_kernel_spmd(
        nc, in_maps, core_ids=list(range(N_CORES)), trace=trace, tmpdir=tmpdir
    )
    LAST_RESULT = res
    outs = [res.results[i]["out"] for i in range(N_CORES)]
    full = np.concatenate(outs, axis=1).astype(np.float32) * (1.0 / OSCALE)
    return full.reshape(-1)
